# revision 43
# baseline (speedup 1.0000x reference)
"""Trainium2 Bass kernel for the BasicRNN problem — pair-data-parallel.

Math: the reference's 9 block matmuls per step collapse to
    state_{s+1} = relu(state_s @ W + gate_s * [E, 0, 0]),  state [256, 4096]
with E = x @ in_w.T + in_b, gates at s % 5 == 0, output = O_T @ out_w.T + out_b
where O is the last 1024 state columns.

Device strategy (8 cores = 4 HBM-domain pairs, measured on this platform):
- Cross-pair collectives cost ~6.2us + 50ns/KB; intra-pair (2-core groups
  [[0,1],[2,3],[4,5],[6,7]]) only ~5.5us + 8.5ns/KB, because pair cores
  share an HBM domain. So: data-parallel over PAIRS (batch 64 per pair),
  W columns split between the two cores of a pair -> zero cross-pair bytes.
- Core role r = core%2 owns the 16 even (r=0) / odd (r=1) 128-row chunks of
  the 4096-dim state, giving each core 4 S + 8 I + 4 O chunks so the E
  injection, last step, and classifier all stay balanced and the SPMD
  program is fully symmetric (per-core behavior differs only via inputs).
- Per step each core computes its 16 output blocks [128,64] = W_own.T @ sT
  (512 matmuls [128,128]x[128,64] bf16, ~34ns each measured back-to-back).
  Blocks 0..b-1 are exchanged as soon as ready (AG1, posted ~halfway into
  the step), blocks b..15 at step end (AG2). The next step consumes AG1
  chunks (phase 1) before AG2 chunks (phases 2/3), so each collective's
  flight hides under matmuls that do not depend on it.
- Consumption order = [member0 blk 0..b-1 | member1 blk 0..b-1 | member0
  blk b.. | member1 blk b..]; per-core W rows are pre-permuted to this
  order on the host, keeping device addressing role-independent.
- ALL dram tensors are partition-major ([128, cols]) so every DMA is an
  identity transfer with >=640B bursts — row-major [N,64] layouts caused
  thousands of 128B bursts and 13us descriptor builds.
- PSUM "start" zeroes a whole 2KB bank (zero region), so the 16 blocks are
  packed 4-per-bank with one accumulation group per bank: start on the
  bank's first matmul, stop on its last.
- Step 1 (state is E-only) contracts just the 8 S chunks and posts one
  combined 16-block AG that rides out the communicator cold start (~60us),
  which itself overlaps the 16.8MB W load.
- Matmul operands bf16 (fp32 is 4x slower), PSUM accumulates fp32.
- Fillers (identity matmuls pinned to live data) bridge residual collective
  latency so the PE p-state stays at 2.4 GHz (idle re-throttles to 1.2).
- Classifier: each core computes 500 classes x its batch-64 from the full
  O (own 4 chunks + partner's via a last small AG); host reassembles.
"""

import numpy as np

S_DIM, I_DIM, O_DIM = 1024, 2048, 1024
TOTAL = 4096
INPUT_DIM, NUM_CLASSES, BATCH = 2048, 1000, 256
NC = 8
CH = 32            # 128-row chunks of the state
BPC = 16           # blocks (chunks) per core
B64 = 64           # batch per pair
BSPLIT = 5         # blocks 0..BSPLIT-1 go in AG1, rest in AG2
CLS_M = 125        # classifier rows per psum block (4 blocks = 500/core)

FILL_WARM = 100    # fillers after E phase (keep PE warm into step 1)
FILL_S = 190       # fillers at each full-step boundary (cover AG1 tail)
FILL_P = 100       # fillers before phase 2 (cover AG2 tail)
STRIP_CC_WAITS = False  # dropping CC trigger waits produced NaNs: the
                        # trigger-level DMA wait IS the input guard

# own chunk list per role, own-order j: j 0..3 = S, 4..11 = I, 12..15 = O
OWN = [[2 * j + r for j in range(BPC)] for r in range(2)]
# consumption order (also per-core W row order and AG output layouts)
A_ORDER = OWN[0][:BSPLIT] + OWN[1][:BSPLIT]
B_ORDER = OWN[0][BSPLIT:] + OWN[1][BSPLIT:]
DEV = A_ORDER + B_ORDER                      # 32 orig chunk ids
POS = {c: i for i, c in enumerate(DEV)}      # orig chunk -> consumption pos
S_POS = [POS[c] for c in OWN[0][:4] + OWN[1][:4]]   # AG0 row order

RG2 = [[0, 1], [2, 3], [4, 5], [6, 7]]

_cache = {}


def _split_excess_waits(nc, mybir, limit=1, nop_limit=1):
    """This walrus build rejects multiple sync-waits on most instruction
    structs and any wait on Drain. Move excess waits onto preceding
    same-engine nops."""
    counter = [0]

    def make_nop(engine, waits):
        counter[0] += 1
        nop = mybir.InstNoOp(name=f"I-ws{counter[0]}", engine=engine)
        nop.sync_info = mybir.SyncInfo(on_wait=list(waits), on_update=[])
        return nop

    for fn in nc.m.functions:
        for bb in fn.blocks:
            out = []
            changed = False
            for inst in bb.instructions:
                si = getattr(inst, "sync_info", None)
                if (
                    STRIP_CC_WAITS
                    and isinstance(inst, mybir.InstCollectiveCompute)
                    and si is not None
                    and si.on_wait
                ):
                    # The CC ucode re-waits its input DMA semaphore itself;
                    # dropping the trigger-level wait lets the CC spin up
                    # (~2us) concurrently with the input DMA.
                    kept = [
                        w for w in si.on_wait
                        if not str(getattr(w, "ant_name", "")).startswith("DMAHW")
                    ]
                    if len(kept) != len(si.on_wait):
                        si.on_wait = kept
                        changed = True
                waits = list(si.on_wait) if si is not None and si.on_wait else []
                lim = 0 if isinstance(inst, mybir.InstDrain) else limit
                if len(waits) > lim:
                    keep = waits[-lim:] if lim else []
                    excess = waits[: len(waits) - lim]
                    for g in range(0, len(excess), nop_limit):
                        nop = make_nop(inst.engine, excess[g : g + nop_limit])
                        nc.register_instruction(nop, overwrite=True)
                        out.append(nop)
                    si.on_wait = keep
                    changed = True
                out.append(inst)
            if changed:
                bb.instructions = out


def _build(T):
    import concourse.bass as bass
    import concourse.tile as tile
    from concourse import mybir

    f32 = mybir.dt.float32
    bf16 = mybir.dt.bfloat16
    b = BSPLIT
    NA = 2 * b              # chunks in AG1 / phase-1
    NB = CH - NA            # chunks in AG2
    HB = BPC - b            # blocks per member in AG2
    CW = 128 * BPC          # sbuf W cols per consumption position

    nc = bass.Bass()
    wc = nc.dram_tensor("wc", [128, CH * CW], bf16, kind="ExternalInput")
    xT = nc.dram_tensor("xT", [128, 16 * B64], bf16, kind="ExternalInput")
    inwTc = nc.dram_tensor("inwTc", [128, 16 * 512], bf16, kind="ExternalInput")
    inbc = nc.dram_tensor("inbc", [128, 4], f32, kind="ExternalInput")
    outwTc = nc.dram_tensor(
        "outwTc", [128, 12 * NUM_CLASSES], bf16, kind="ExternalInput"
    )
    outbc = nc.dram_tensor("outbc", [CLS_M, 8], f32, kind="ExternalInput")
    ident = nc.dram_tensor("ident", [128, 128], bf16, kind="ExternalInput")
    out_t = nc.dram_tensor("out_t", [CLS_M, 8 * B64], f32, kind="ExternalOutput")

    def ag(ins_ap, out_ap):
        nc.gpsimd.collective_compute(
            "AllGather", mybir.AluOpType.bypass,
            replica_groups=RG2, ins=[ins_ap], outs=[out_ap],
        )

    with tile.TileContext(nc) as tc:
        with (
            tc.tile_pool(name="wp", bufs=1) as wp,
            tc.tile_pool(name="pers", bufs=1) as pers,
            tc.tile_pool(name="state", bufs=2) as stp,
            tc.tile_pool(name="res", bufs=2) as resp,
            tc.tile_pool(name="psum", bufs=1, space="PSUM") as psp,
            tc.tile_pool(name="psum1", bufs=1, space="PSUM") as psp1,
            tc.tile_pool(name="dram", bufs=2, space="DRAM") as dram,
        ):
            # --- small consts + E-phase inputs first ---
            id_t = pers.tile([128, 128], bf16, name="ident", tag="ident")
            nc.sync.dma_start(id_t[:], ident[:])
            inb_t = pers.tile([128, 4], f32, name="inb", tag="inb")
            nc.sync.dma_start(inb_t[:], inbc[:])
            outb_t = pers.tile([CLS_M, 8], f32, name="outb", tag="outb")
            nc.sync.dma_start(outb_t[:], outbc[:])
            iwt = pers.tile([128, 16 * 512], bf16, name="iwt", tag="iwt")
            nc.sync.dma_start(iwt[:], inwTc[:])
            xt = pers.tile([128, 16 * B64], bf16, name="xt", tag="xt")
            nc.scalar.dma_start(xt[:], xT[:])

            # --- E phase: own 4 S blocks of E = in_w @ x.T + in_b  [128,64]
            # NOTE: psum "start" zeroes the whole 2KB zero region (bank), so a
            # packed tile must be one accumulation group: start only on the
            # very first matmul into the bank, stop only on the last.
            ps_e = psp1.tile([128, 4 * B64], f32, name="pse", tag="pse")
            KE = INPUT_DIM // 128
            for k in range(KE):
                for j in range(4):
                    nc.tensor.matmul(
                        ps_e[:, B64 * j : B64 * (j + 1)],
                        iwt[:, k * 512 + 128 * j : k * 512 + 128 * (j + 1)],
                        xt[:, B64 * k : B64 * (k + 1)],
                        start=(k == 0 and j == 0), stop=(k == KE - 1 and j == 3),
                    )
            einj = pers.tile([128, 4 * B64], bf16, name="einj", tag="einj")
            st1own = pers.tile([128, 4 * B64], bf16, name="st1own", tag="st1own")
            for j in range(4):
                sl = slice(B64 * j, B64 * (j + 1))
                nc.scalar.activation(
                    einj[:, sl], ps_e[:, sl],
                    mybir.ActivationFunctionType.Identity, bias=inb_t[:, j : j + 1],
                )
                nc.scalar.activation(
                    st1own[:, sl], ps_e[:, sl],
                    mybir.ActivationFunctionType.Relu, bias=inb_t[:, j : j + 1],
                )
            # AG0 is the FIRST collective: it absorbs the communicator cold
            # start (NEFF launch stagger across cores), which overlaps the W
            # load below.
            agin0 = dram.tile([128, 4 * B64], bf16, name="agin0", tag="agin0")
            nc.sync.dma_start(agin0[:], st1own[:])
            agout0 = dram.tile([256, 4 * B64], bf16, name="agout0", tag="agout0")
            ag(agin0.opt(), agout0.opt())

            # --- big W load (16.8MB). Both halves are queued BEHIND a DMA
            # that depends on the E phase, so the E inputs always win the
            # shared DMA rings; the W data then overlaps the communicator
            # cold start. ---
            gate = dram.tile([1, B64], bf16, name="gate", tag="gate")
            nc.scalar.dma_start(gate[:], st1own[0:1, 0:B64])
            wt = wp.tile([128, CH * CW], bf16, name="wt", tag="wt")
            HW = CH * CW // 2
            nc.sync.dma_start(wt[:, :HW], wc[:, :HW])
            nc.scalar.dma_start(wt[:, HW:], wc[:, HW:])
            owt = pers.tile(
                [128, 12 * NUM_CLASSES], bf16, name="owt", tag="owt"
            )
            nc.scalar.dma_start(owt[:], outwTc[:])

            def wslice(k, m):  # lhsT tile [128,128] for (consumption pos, block)
                off = k * CW + m * 128
                return wt[:, off : off + 128]

            ps_d = psp1.tile([128, B64], f32, name="psd", tag="psd")

            def fill(n, rhs):  # keep PE busy/warm through a collective wait
                # rhs pins the fillers to the producing step: without a data
                # dep the scheduler drains every filler into the first
                # simulated bubble.
                for _ in range(n):
                    nc.tensor.matmul(
                        ps_d[:], id_t[:], rhs[:, 0:B64], start=True, stop=True
                    )

            fill(FILL_WARM, einj)

            # --- RNN steps s = 1..T-1 ---
            last = T - 1
            res = None
            sta0 = sta1 = stb0 = stb1 = None
            for s in range(1, T):
                inject = (s % 5 == 0) and s != last
                m_list = list(range(12, 16)) if s == last else list(range(BPC))

                # state tiles for this step (loaded at the end of step s-1)
                if s == 1:
                    st0 = stp.tile([128, 8 * B64], bf16, name="st0", tag="st0")
                    nc.sync.dma_start(st0[:, : 4 * B64], agout0[0:128, :])
                    nc.scalar.dma_start(st0[:, 4 * B64 :], agout0[128:256, :])
                    chunks = [(st0, i, S_POS[i]) for i in range(8)]
                    ka = len(chunks)
                else:
                    chunks = [
                        (sta0, i, i) if i < b else (sta1, i - b, i)
                        for i in range(NA)
                    ] + [
                        (stb0, i, NA + i) if i < HB else (stb1, i - HB, NA + i)
                        for i in range(NB)
                    ]
                    ka = NA

                ps = [
                    psp.tile([128, 4 * B64], f32, name=f"ps{g}", tag=f"ps{g}")
                    for g in range(4)
                ]

                def pslice(m):
                    g, u = divmod(m, 4)
                    return ps[g][:, B64 * u : B64 * (u + 1)]

                def do_mm(kk, m, start, stop):
                    t, i, k = chunks[kk]
                    nc.tensor.matmul(
                        pslice(m), wslice(k, m),
                        t[:, B64 * i : B64 * (i + 1)],
                        start=start, stop=stop,
                    )

                nk = len(chunks)
                res = resp.tile([128, BPC * B64], bf16, name="res", tag="res")

                def finish(m):  # close accumulation, relu, into res
                    eng = nc.vector.tensor_relu if m % 2 == 0 else (
                        lambda o, i_: nc.scalar.activation(
                            o, i_, mybir.ActivationFunctionType.Relu
                        )
                    )
                    eng(res[:, B64 * m : B64 * (m + 1)], pslice(m))

                # phase 1: A chunks x all blocks (s==1: the whole contraction)
                for kk in range(ka):
                    for m in m_list:
                        do_mm(
                            kk, m,
                            kk == 0 and m % 4 == 0,
                            ka == nk and kk == nk - 1 and m % 4 == 3,
                        )
                if s != 1:
                    fill(FILL_P if s != last else 175, sta0)
                # phase 2: B chunks x blocks 0..b-1 (or all of last step).
                # m-outer: blocks complete staggered so their relus overlap
                # the remaining matmuls instead of serializing at phase end.
                m2 = [m for m in m_list if m < b] if s != last else m_list
                m3 = [m for m in m_list if m >= b] if s != last else []
                for kk in range(ka, nk):
                    for m in m2:
                        do_mm(
                            kk, m, False,
                            kk == nk - 1 and m % 4 == 3
                            and not (inject and m < 4),
                        )
                for m in m2:
                    if inject and m < 4:
                        nc.tensor.matmul(
                            pslice(m), id_t[:],
                            einj[:, B64 * m : B64 * (m + 1)],
                            start=False, stop=(m == 3),
                        )
                    finish(m)
                if s == last:
                    break
                agin1 = dram.tile([128, b * B64], bf16, name="agin1", tag="agin1")
                nc.sync.dma_start(agin1[:], res[:, : b * B64])
                agout1 = dram.tile([256, b * B64], bf16, name="agout1", tag="agout1")
                ag(agin1.opt(), agout1.opt())
                # phase 3: B chunks x blocks b..15
                for kk in range(ka, nk):
                    for m in m3:
                        do_mm(kk, m, False, kk == nk - 1 and m % 4 == 3)
                for m in m3:
                    finish(m)
                agin2 = dram.tile([128, HB * B64], bf16, name="agin2", tag="agin2")
                nc.scalar.dma_start(agin2[:], res[:, b * B64 :])
                agout2 = dram.tile(
                    [256, HB * B64], bf16, name="agout2", tag="agout2"
                )
                ag(agin2.opt(), agout2.opt())

                # prefetch next step's state tiles (wait on the AGs)
                sta0 = stp.tile([128, b * B64], bf16, name="sta0", tag="sta0")
                sta1 = stp.tile([128, b * B64], bf16, name="sta1", tag="sta1")
                stb0 = stp.tile([128, HB * B64], bf16, name="stb0", tag="stb0")
                stb1 = stp.tile([128, HB * B64], bf16, name="stb1", tag="stb1")
                nc.sync.dma_start(sta0[:], agout1[0:128, :])
                nc.scalar.dma_start(sta1[:], agout1[128:256, :])
                nc.sync.dma_start(stb0[:], agout2[0:128, :])
                nc.scalar.dma_start(stb1[:], agout2[128:256, :])
                fill(FILL_S, res)

            # --- classifier: exchange O halves (64KB), contract own chunks
            # from res immediately (phase A) and both AG halves with the own
            # member's rows zeroed in owt (phase B) — fully symmetric ---
            agin3 = dram.tile([128, 4 * B64], bf16, name="agin3", tag="agin3")
            nc.sync.dma_start(agin3[:], res[:, 12 * B64 :])
            agout3 = dram.tile([256, 4 * B64], bf16, name="agout3", tag="agout3")
            ag(agin3.opt(), agout3.opt())
            ps_c = psp1.tile([CLS_M, 8 * B64], f32, name="psc", tag="psc")

            def cls_mm(q, cb, rhs, start, stop):
                nc.tensor.matmul(
                    ps_c[:, B64 * cb : B64 * (cb + 1)],
                    owt[:, q * NUM_CLASSES + CLS_M * cb :
                        q * NUM_CLASSES + CLS_M * (cb + 1)],
                    rhs, start=start, stop=stop,
                )

            for k in range(4):  # phase A: own O chunks straight from res
                for cb in range(8):
                    cls_mm(
                        k, cb, res[:, B64 * (12 + k) : B64 * (13 + k)],
                        k == 0 and cb == 0, False,
                    )
            # pin to a res slice the LAST step actually wrote (blocks 12..15)
            fill(230, res[:, 12 * B64 :])
            ot = stp.tile([128, 8 * B64], bf16, name="ot", tag="ot")
            nc.sync.dma_start(ot[:, : 4 * B64], agout3[0:128, :])
            nc.scalar.dma_start(ot[:, 4 * B64 :], agout3[128:256, :])
            for k2 in range(8):  # phase B: both AG halves, own rows zeroed
                for cb in range(8):
                    cls_mm(
                        4 + k2, cb, ot[:, B64 * k2 : B64 * (k2 + 1)],
                        False, k2 == 7 and cb == 7,
                    )
            out_sb = pers.tile([CLS_M, 8 * B64], f32, name="outsb", tag="outsb")
            for cb in range(8):
                sl = slice(B64 * cb, B64 * (cb + 1))
                nc.scalar.activation(
                    out_sb[:, sl], ps_c[:, sl],
                    mybir.ActivationFunctionType.Identity,
                    bias=outb_t[:, cb : cb + 1],
                )
            nc.sync.dma_start(out_t[:], out_sb[:])

    _split_excess_waits(nc, mybir)
    return nc


def _tile_pm(a):
    """[R, C] -> partition-major [128, (R//128)*C]: out[p, k*C+c] = a[128k+p, c]."""
    r, c = a.shape
    return np.ascontiguousarray(
        a.reshape(r // 128, 128, c).transpose(1, 0, 2).reshape(128, -1)
    )


def kernel(x, W, in_w, in_b, out_w, out_b, time_steps):
    T = int(time_steps)
    x = np.ascontiguousarray(x, dtype=np.float32)
    W = np.ascontiguousarray(W, dtype=np.float32)
    in_w = np.ascontiguousarray(in_w, dtype=np.float32)
    in_b = np.ascontiguousarray(in_b, dtype=np.float32)
    out_w = np.ascontiguousarray(out_w, dtype=np.float32)
    out_b = np.ascontiguousarray(out_b, dtype=np.float32)

    if T < 2:
        # T=0: O stays 0; T=1: state_1 = [relu(E),0,0], O still 0.
        return np.broadcast_to(out_b, (BATCH, NUM_CLASSES)).astype(np.float32).copy()

    import ml_dtypes
    from concourse.bass_utils import run_bass_kernel_spmd

    if T not in _cache:
        _cache[T] = _build(T)
    nc = _cache[T]

    bf = ml_dtypes.bfloat16
    row_perm = np.concatenate([np.arange(128 * c, 128 * c + 128) for c in DEV])
    Wd = W[row_perm].astype(bf)  # [4096, 4096], rows in consumption order
    xTa = np.ascontiguousarray(x.T.astype(bf))
    inwT = in_w.T.astype(bf)
    outwT = out_w.T.astype(bf)
    ident = np.eye(128, dtype=np.float32).astype(bf)
    outb_d = np.ascontiguousarray(out_b.reshape(8, CLS_M).T.copy())

    in_maps = []
    for c in range(NC):
        p, r = divmod(c, 2)
        own_cols = np.concatenate(
            [np.arange(128 * ch, 128 * ch + 128) for ch in OWN[r]]
        )
        own_s_rows = np.concatenate(
            [np.arange(128 * ch, 128 * ch + 128) for ch in OWN[r][:4]]
        )
        # classifier weights: [own 4 O chunks | AG3-order 8 chunks with the
        # own member's rows zeroed] -> [1536, 1000]
        o_parts = [outwT[128 * (ch - 24) : 128 * (ch - 24) + 128]
                   for ch in OWN[r][12:]]
        for m in range(2):
            for ch in OWN[m][12:]:
                blkw = outwT[128 * (ch - 24) : 128 * (ch - 24) + 128]
                o_parts.append(np.zeros_like(blkw) if m == r else blkw)
        owtd = np.concatenate(o_parts, axis=0)
        in_maps.append({
            "wc": _tile_pm(Wd[:, own_cols]),
            "xT": _tile_pm(xTa[:, B64 * p : B64 * (p + 1)]),
            "inwTc": _tile_pm(inwT[:, own_s_rows]),
            "inbc": np.ascontiguousarray(
                in_b[own_s_rows].reshape(4, 128).T.copy()
            ),
            "outwTc": _tile_pm(owtd),
            "outbc": outb_d,
            "ident": ident,
        })
    res = run_bass_kernel_spmd(nc, in_maps, list(range(NC)))
    out = np.empty((BATCH, NUM_CLASSES), dtype=np.float32)
    for p in range(4):
        # even core of each pair holds the full summed logits for its batch
        arr = res.results[2 * p]["out_t"].reshape(CLS_M, 8, B64)
        blk = arr.transpose(1, 0, 2).reshape(NUM_CLASSES, B64)
        out[B64 * p : B64 * (p + 1), :] = blk.T
    return out
